# revision 16
# baseline (speedup 1.0000x reference)
"""Trainium2 Bass kernel for nn_MultiHeadAttention_84791244358011.

Linear (ELU feature-map) attention:
    x_norm = LayerNorm(x)                      # eps=1e-12
    q = x_norm @ Wq.T + bq ; k,v = x @ W.T + b # per-head [S, 64]
    eq/ek = l2norm(elu(q/k)) per token over head_dim
    kv = ek^T @ v per head [64, 64]; ctx = eq @ kv / 8
    out = ctx @ Wo.T + bo + x

Sharding: data-parallel over batch B=8 — one batch element per NeuronCore,
no collectives. All matmul operands are bf16 (PSUM accumulation stays f32).

Single pass over the 32 token tiles (vs. the old two-pass + DRAM spill):
  - x^T comes straight from DRAM through the DMA xbar transpose (one
    InstDmaTransposeAnt per tile) — no PE transposes, no PSUM copies.
  - LayerNorm is folded into the Q projection:
        q = rstd * (x @ wqt - mu x rw) + bq_eff,   rw_j = sum_i wqt[i,j]
    mu enters as a rank-1 matmul (mu row x -rw row); rstd enters as the
    per-partition *scale* operand of the ELU's two Relu activations.
    The mu row itself is a ones-column matmul against x^T, borrowed into
    the q0 PSUM bank before the q accumulation starts.
  - elu(x)+1 == relu(x) + exp(min(x,0)) exactly, so the ELU is
    relu(-x) -> exp(-.) on ACT, relu(x) on ACT, one add on Pool; the
    "-1" folds into Square's bias (sum-of-squares) and a tensor_scalar.
  - all rsqrt via exp(-0.5*ln(.)) on ACT: ln and exp live in the same
    activation table set (natural_log_exp_and_others) so there are no
    1.28us table swaps, and no DVE Newton chains.
  - the k-side l2 norm is applied to V instead of K (kv = sum ek x v is
    bilinear), saving a broadcast multiply from PSUM.
  - per-head kv state accumulates into a single PSUM bank: even heads at
    partitions 0-63, odd heads at 64-127 via tile_position=(0,64).
  - eq is written bf16 and xbar-transposed SBUF->SBUF into a persistent
    [128, 8, 4096] eqT buffer; pass 2 (ctx = kv @ eqT, out = ctx^T @ wot
    + x) runs from SBUF with no transposes.
  - kv accumulation matmuls for tile t issue after tile t+1's projection
    matmuls so the PE never waits on the k-norm chain.

Biases are all zero for this problem's inputs; a with_bias variant adds
rank-1 (ones x bias) matmuls into each projection's PSUM group.
"""

import contextlib

import numpy as np

import concourse.bass as bass
import concourse.mybir as mybir
import concourse.tile as tile
from concourse import bacc

B, S, HID = 8, 4096, 1024
NH, HD = 16, 64
P = 128
NT = S // P            # 32 token tiles
NC = HID // P          # 8 feature chunks
CHUNK = 4              # token tiles per ctx chunk (512 tokens)
NCHUNKS = NT // CHUNK
LN_EPS = 1e-12

F32 = mybir.dt.float32
BF16 = mybir.dt.bfloat16
AF = mybir.ActivationFunctionType
OP = mybir.AluOpType
AX = mybir.AxisListType


def build_nc(with_bias=False, loop_n=1):
    nc = bacc.Bacc("TRN2", target_bir_lowering=False, enable_partition_id=False)

    x_d = nc.dram_tensor("x", [S, HID], BF16, kind="ExternalInput")
    wqt_d = nc.dram_tensor("wqt", [HID, HID], BF16, kind="ExternalInput")
    wkt_d = nc.dram_tensor("wkt", [HID, HID], BF16, kind="ExternalInput")
    wvt_d = nc.dram_tensor("wvt", [HID, HID], BF16, kind="ExternalInput")
    wot_d = nc.dram_tensor("wot", [HID, HID], BF16, kind="ExternalInput")
    nrw_d = nc.dram_tensor("nrw", [1, HID], BF16, kind="ExternalInput")
    bias_d = {}
    if with_bias:
        for nm in ("bq", "bk", "bv", "bo"):
            bias_d[nm] = nc.dram_tensor(nm, [1, HID], BF16, kind="ExternalInput")
    out_d = nc.dram_tensor("out", [S, HID], F32, kind="ExternalOutput")

    with tile.TileContext(nc) as tc, contextlib.ExitStack() as ctx:
        persist = ctx.enter_context(tc.tile_pool(name="persist", bufs=1))
        wpool = ctx.enter_context(tc.tile_pool(name="weights", bufs=1))

        ones_col = persist.tile([P, 1], BF16)
        nc.gpsimd.memset(ones_col, 1.0)
        eps_c = persist.tile([P, 1], F32)
        nc.gpsimd.memset(eps_c, LN_EPS)
        negone_c = persist.tile([P, 1], F32)
        nc.gpsimd.memset(negone_c, -1.0)
        kv_sb = persist.tile([P, NC * HD], BF16)      # packed kv state
        eqT = persist.tile([P, NC, S], BF16)          # transposed eq, full S
        nrw_sb = persist.tile([1, HID], BF16)
        nc.sync.dma_start(nrw_sb, nrw_d.ap())
        brow = {}
        if with_bias:
            ones_row = persist.tile([1, P], BF16)
            nc.gpsimd.memset(ones_row, 1.0)
            for nm, d in bias_d.items():
                t_ = persist.tile([1, HID], BF16, name=f"{nm}_row")
                nc.sync.dma_start(t_, d.ap())
                brow[nm] = t_

        _loop = tc.For_i(0, loop_n, 1) if loop_n > 1 else contextlib.nullcontext(0)
        with _loop:
            # ------------- pass 1: k/v/q, kv state, eqT -------------
            wk_sb = wpool.tile([P, NC, HID], BF16, tag="wA", name="wk_sb")
            nc.gpsimd.dma_start(wk_sb, wkt_d.ap().rearrange("(c p) j -> p c j", p=P))
            wv_sb = wpool.tile([P, NC, HID], BF16, tag="wB", name="wv_sb")
            nc.gpsimd.dma_start(wv_sb, wvt_d.ap().rearrange("(c p) j -> p c j", p=P))
            wq_sb = wpool.tile([P, NC, HID], BF16, tag="wC", name="wq_sb")
            nc.gpsimd.dma_start(wq_sb, wqt_d.ap().rearrange("(c p) j -> p c j", p=P))
            wo_sb = wpool.tile([P, NC, HID], BF16, tag="wD", name="wo_sb")
            nc.gpsimd.dma_start(wo_sb, wot_d.ap().rearrange("(c p) j -> p c j", p=P))

            with tc.tile_pool(name="sbufA", bufs=1) as sa, \
                 tc.tile_pool(name="psumA", bufs=1, space="PSUM") as pa:
                kv_ps = pa.tile([P, NC * HD], F32, tag="kv", name="kv_ps")
                pend_kv = []           # deferred kv matmuls (ek, vs) per tile

                def flush_kv(last):
                    if not pend_kv:
                        return
                    first, ek, vs = pend_kv.pop()
                    for a in range(NC):
                        nc.tensor.matmul(
                            kv_ps[0:HD, a * HD:(a + 1) * HD],
                            vs[:, 2 * a, :], ek[:, 2 * a, :],
                            start=(first and a == 0), stop=last,
                            tile_position=(0, 0), skip_group_check=True)
                        nc.tensor.matmul(
                            kv_ps[HD:P, a * HD:(a + 1) * HD],
                            vs[:, 2 * a + 1, :], ek[:, 2 * a + 1, :],
                            start=False, stop=last,
                            tile_position=(0, 64), skip_group_check=True)

                for t in range(NT):
                    tok = slice(t * P, (t + 1) * P)
                    x_t = sa.tile([P, HID], BF16, tag="x", bufs=3, name=f"x_{t}")
                    nc.scalar.dma_start(x_t, x_d.ap()[tok, :])
                    xT = sa.tile([P, NC, P], BF16, tag="xT", bufs=3,
                                 name=f"xT_{t}")
                    nc.sync.dma_start_transpose(xT, x_d.ap()[tok, :])

                    # LayerNorm stats; rstd = exp(-0.5*ln(var+eps))
                    stats = sa.tile([P, 2, 6], F32, tag="st", bufs=3,
                                    name=f"st_{t}")
                    xg = x_t[:].rearrange("p (g d) -> p g d", g=2)
                    for g in range(2):
                        nc.vector.bn_stats(stats[:, g, :], xg[:, g, :])
                    mv = sa.tile([P, 2], F32, tag="mv", bufs=3, name=f"mv_{t}")
                    nc.vector.bn_aggr(mv, stats)
                    lnv = sa.tile([P, 1], F32, tag="lnv", bufs=3, name=f"lnv_{t}")
                    nc.scalar.activation(lnv, mv[:, 1:2], AF.Ln, bias=eps_c[:, 0:1])
                    rstd = sa.tile([P, 1], F32, tag="rsd", bufs=3, name=f"rsd_{t}")
                    nc.scalar.activation(rstd, lnv, AF.Exp, scale=-0.5)
                    nrstd = sa.tile([P, 1], F32, tag="nrs", bufs=3, name=f"nrs_{t}")
                    nc.vector.tensor_scalar(nrstd, rstd, -1.0, None, OP.mult)

                    def proj(ps, w_sb, half, extras=()):
                        sl = slice(half * 512, (half + 1) * 512)
                        for c in range(NC):
                            nc.tensor.matmul(
                                ps, xT[:, c, :], w_sb[:, c, sl],
                                start=(c == 0),
                                stop=(c == NC - 1 and not extras),
                                skip_group_check=True)
                        for i, (lhs, rhs_row) in enumerate(extras):
                            nc.tensor.matmul(
                                ps, lhs, rhs_row[0:1, sl],
                                start=False, stop=(i == len(extras) - 1),
                                skip_group_check=True)

                    # kv accumulation for the previous tile (PE never waits)
                    flush_kv(last=False)

                    # ---- K ----
                    k_ps = [pa.tile([P, 512], F32, tag="kh", bufs=2,
                                    name=f"k_ps{t}_{h}") for h in range(2)]
                    kex = [(ones_row, brow["bk"])] if with_bias else []
                    w1k = sa.tile([P, HID], BF16, tag="w1k", bufs=3,
                                  name=f"w1k_{t}")
                    for half in range(2):
                        proj(k_ps[half], wk_sb, half, kex)
                        hs = slice(half * 512, (half + 1) * 512)
                        r = sa.tile([P, 512], BF16, tag="kr", bufs=3,
                                    name=f"kr_{t}_{half}")
                        nc.scalar.activation(r, k_ps[half], AF.Relu, scale=-1.0)
                        e = sa.tile([P, 512], BF16, tag="ke", bufs=3,
                                    name=f"ke_{t}_{half}")
                        nc.scalar.activation(e, r, AF.Exp, scale=-1.0)
                        m = sa.tile([P, 512], BF16, tag="km", bufs=3,
                                    name=f"km_{t}_{half}")
                        nc.scalar.activation(m, k_ps[half], AF.Relu)
                        nc.gpsimd.tensor_tensor(w1k[:, hs], m, e, OP.add)
                    sqk = sa.tile([P, NH, HD], BF16, tag="sqk", bufs=2,
                                  name=f"sqk_{t}")
                    nc.scalar.activation(
                        sqk[:].rearrange("p h d -> p (h d)"), w1k, AF.Square,
                        bias=negone_c[:, 0:1])
                    ssk = sa.tile([P, NH], F32, tag="ssk", bufs=3,
                                  name=f"ssk_{t}")
                    nc.vector.tensor_reduce(ssk, sqk, AX.X, OP.add)
                    lnk = sa.tile([P, NH], F32, tag="lnk", bufs=3,
                                  name=f"lnk_{t}")
                    nc.scalar.activation(lnk, ssk, AF.Ln)
                    rnk = sa.tile([P, NH], F32, tag="rnk", bufs=3,
                                  name=f"rnk_{t}")
                    nc.scalar.activation(rnk, lnk, AF.Exp, scale=-0.5)
                    ek = sa.tile([P, NH, HD], BF16, tag="ek", bufs=2,
                                 name=f"ek_{t}")
                    nc.vector.tensor_scalar(
                        ek[:].rearrange("p h d -> p (h d)"), w1k, 1.0, None,
                        OP.subtract)

                    # ---- V (k-norm folded in) ----
                    v_ps = [pa.tile([P, 512], F32, tag="vh", bufs=3,
                                    name=f"v_ps{t}_{h}") for h in range(2)]
                    vex = [(ones_row, brow["bv"])] if with_bias else []
                    vs = sa.tile([P, NH, HD], BF16, tag="vs", bufs=2,
                                 name=f"vs_{t}")
                    for half in range(2):
                        proj(v_ps[half], wv_sb, half, vex)
                        hh = slice(half * 8, (half + 1) * 8)
                        nc.vector.tensor_tensor(
                            vs[:, hh, :],
                            v_ps[half][:].rearrange("p (h d) -> p h d", d=HD),
                            rnk[:, hh, None].to_broadcast((P, 8, HD)),
                            OP.mult)

                    # mu row via ones-matmul, borrowed into the q0 psum bank
                    q_ps = [pa.tile([P, 512], F32, tag="qh", bufs=2,
                                    name=f"q_ps{t}_{h}") for h in range(2)]
                    for c in range(NC):
                        nc.tensor.matmul(q_ps[0][0:1, 0:P], ones_col,
                                         xT[:, c, :],
                                         start=(c == 0), stop=(c == NC - 1),
                                         skip_group_check=True)
                    mu_row = sa.tile([1, P], BF16, tag="mu", bufs=3,
                                     name=f"mu_{t}")
                    nc.vector.tensor_scalar(mu_row, q_ps[0][0:1, 0:P],
                                            1.0 / HID, None, OP.mult)

                    # ---- Q (LayerNorm folded in) ----
                    qex = [(mu_row, nrw_sb)]
                    if with_bias:
                        qex.append((ones_row, brow["bq"]))
                    w1q = sa.tile([P, HID], BF16, tag="w1q", bufs=3,
                                  name=f"w1q_{t}")
                    for half in range(2):
                        proj(q_ps[half], wq_sb, half, qex)
                        hs = slice(half * 512, (half + 1) * 512)
                        r = sa.tile([P, 512], BF16, tag="qr", bufs=3,
                                    name=f"qr_{t}_{half}")
                        nc.scalar.activation(r, q_ps[half], AF.Relu,
                                             scale=nrstd[:, 0:1])
                        e = sa.tile([P, 512], BF16, tag="qe", bufs=3,
                                    name=f"qe_{t}_{half}")
                        nc.scalar.activation(e, r, AF.Exp, scale=-1.0)
                        m = sa.tile([P, 512], BF16, tag="qm", bufs=3,
                                    name=f"qm_{t}_{half}")
                        nc.scalar.activation(m, q_ps[half], AF.Relu,
                                             scale=rstd[:, 0:1])
                        nc.gpsimd.tensor_tensor(w1q[:, hs], m, e, OP.add)
                    sqq = sa.tile([P, NH, HD], BF16, tag="sqq", bufs=2,
                                  name=f"sqq_{t}")
                    nc.scalar.activation(
                        sqq[:].rearrange("p h d -> p (h d)"), w1q, AF.Square,
                        bias=negone_c[:, 0:1])
                    ssq = sa.tile([P, NH], F32, tag="ssq", bufs=3,
                                  name=f"ssq_{t}")
                    nc.vector.tensor_reduce(ssq, sqq, AX.X, OP.add)
                    lnq = sa.tile([P, NH], F32, tag="lnq", bufs=3,
                                  name=f"lnq_{t}")
                    nc.scalar.activation(lnq, ssq, AF.Ln)
                    rnq = sa.tile([P, NH], BF16, tag="rnq", bufs=3,
                                  name=f"rnq_{t}")
                    nc.scalar.activation(rnq, lnq, AF.Exp, scale=-0.5)
                    eqp = sa.tile([P, NH, HD], BF16, tag="eqp", bufs=2,
                                  name=f"eqp_{t}")
                    nc.vector.tensor_scalar(
                        eqp[:].rearrange("p h d -> p (h d)"), w1q, 1.0, None,
                        OP.subtract)
                    eq = sa.tile([P, NH, HD], BF16, tag="eq", bufs=2,
                                 name=f"eq_{t}")
                    nc.vector.tensor_tensor(
                        eq, eqp, rnq[:, :, None].to_broadcast((P, NH, HD)),
                        OP.mult)
                    nc.sync.dma_start_transpose(
                        eqT[:, :, tok], eq[:].rearrange("p h d -> p (h d)"))

                    pend_kv.append((t == 0, ek, vs))

                flush_kv(last=True)
                nc.vector.tensor_copy(kv_sb, kv_ps)

            # ------------- pass 2: out = eq @ (blockdiag(kv) @ wot) + x --
            with tc.tile_pool(name="sbufB", bufs=1) as sb, \
                 tc.tile_pool(name="psumB", bufs=1, space="PSUM") as pb:
                # M[c-chunk][p, j]: rows = (head, d) feature index c*128+p.
                # vk_sb holds per-head vk[e, d]: head 2a at partitions 0-63,
                # col block a; head 2a+1 at partitions 64-127, col block a.
                m_sb = sb.tile([P, NC, HID], BF16, tag="m", bufs=1, name="m_sb")
                for c in range(NC):
                    for half in range(2):
                        hs = slice(half * 512, (half + 1) * 512)
                        m_ps = pb.tile([P, 512], F32, tag="mps", bufs=2,
                                       name=f"m_ps{c}_{half}")
                        nc.tensor.matmul(
                            m_ps[0:HD, :], kv_sb[0:HD, c * HD:(c + 1) * HD],
                            wo_sb[0:HD, c, hs], start=True, stop=True,
                            tile_position=(0, 0), skip_group_check=True)
                        nc.tensor.matmul(
                            m_ps[HD:P, :], kv_sb[HD:P, c * HD:(c + 1) * HD],
                            wo_sb[HD:P, c, hs], start=True, stop=True,
                            tile_position=(64, 64), skip_group_check=True)
                        if half == 0:
                            nc.scalar.copy(m_sb[:, c, hs], m_ps)
                        else:
                            nc.vector.tensor_copy(m_sb[:, c, hs], m_ps)

                for t in range(NT):
                    tok = slice(t * P, (t + 1) * P)
                    x2 = sb.tile([P, HID], BF16, tag="x2", bufs=2,
                                 name=f"x2_{t}")
                    nc.scalar.dma_start(x2, x_d.ap()[tok, :])
                    out_sb = sb.tile([P, HID], F32, tag="osb", bufs=2,
                                     name=f"out_{t}")
                    for half in range(2):
                        hs = slice(half * 512, (half + 1) * 512)
                        o_ps = pb.tile([P, 512], F32, tag="oh", bufs=3,
                                       name=f"o_ps{t}_{half}")
                        for c in range(NC):
                            nc.tensor.matmul(
                                o_ps, eqT[:, c, tok], m_sb[:, c, hs],
                                start=(c == 0),
                                stop=(c == NC - 1 and not with_bias),
                                skip_group_check=True)
                        if with_bias:
                            nc.tensor.matmul(
                                o_ps, ones_row, brow["bo"][0:1, hs],
                                start=False, stop=True,
                                skip_group_check=True)
                        nc.vector.tensor_tensor(
                            out_sb[:, hs], o_ps, x2[:, hs], OP.add)
                    nc.gpsimd.dma_start(out_d.ap()[tok, :], out_sb)

    nc.compile()
    _force_single_act_table(nc)
    return nc


def _force_single_act_table(nc):
    """Retarget all activation-table loads to natural_log_exp_and_others
    (which contains every function this kernel uses: ln, exp, relu, square,
    copy) and drop the now-redundant reloads. The insertion pass picks the
    first table containing each function, which thrashes exp<->ln at 1.28us
    per reload, ~6x per token tile."""
    from concourse.hw_specs import get_activation_tables

    names = list(get_activation_tables(nc.m.arch).keys())
    target = names.index("natural_log_exp_and_others")
    kept_one = False
    for b in nc.main_func.blocks:
        keep = []
        for i in b.instructions:
            if isinstance(i, mybir.InstLoadActFuncSet):
                si = getattr(i, "sync_info", None)
                has_sync = si is not None and (
                    len(si.on_wait) > 0 or len(si.on_update) > 0)
                i.act_func_set_id = target
                if not kept_one or has_sync:
                    kept_one = True
                    keep.append(i)
            else:
                keep.append(i)
        b.instructions[:] = keep


_RUNNER = {}


def _get_runner(loop_n=1, with_bias=False):
    key = (loop_n, with_bias)
    if key in _RUNNER:
        return _RUNNER[key]

    import jax
    from jax.sharding import Mesh, PartitionSpec
    from jax.experimental.shard_map import shard_map
    from concourse.bass2jax import _bass_exec_p, install_neuronx_cc_hook

    install_neuronx_cc_hook()
    nc = build_nc(with_bias=with_bias, loop_n=loop_n)

    in_names = []
    out_names = []
    out_avals = []
    for alloc in nc.m.functions[0].allocations:
        if not isinstance(alloc, mybir.MemoryLocationSet):
            continue
        name = alloc.memorylocations[0].name
        if alloc.kind == "ExternalInput":
            in_names.append(name)
        elif alloc.kind == "ExternalOutput":
            out_names.append(name)
            out_avals.append(
                jax.core.ShapedArray(tuple(alloc.tensor_shape),
                                     mybir.dt.np(alloc.dtype)))
    n_params = len(in_names)
    all_in_names = in_names + out_names

    def _body(*args):
        outs = _bass_exec_p.bind(
            *args,
            out_avals=tuple(out_avals),
            in_names=tuple(all_in_names),
            out_names=tuple(out_names),
            lowering_input_output_aliases=(),
            sim_require_finite=True,
            sim_require_nnan=True,
            nc=nc,
        )
        return tuple(outs)

    devices = jax.devices()[:B]
    mesh = Mesh(np.asarray(devices), ("core",))
    n_outs = len(out_names)
    fn = jax.jit(
        shard_map(
            _body, mesh=mesh,
            in_specs=(PartitionSpec("core"),) * (n_params + n_outs),
            out_specs=(PartitionSpec("core"),) * n_outs,
            check_rep=False,
        ),
        keep_unused=True,
    )
    _RUNNER[key] = (fn, in_names, out_names, out_avals)
    return _RUNNER[key]


def prep_inputs(input_tensor, attention_mask, ln_gamma, ln_beta,
                Wq, bq, Wk, bk, Wv, bv, Wo, bo):
    """Host-side static prep: transpose weights, fold gamma/beta/scale."""
    import ml_dtypes
    bf = ml_dtypes.bfloat16
    f = np.float32
    x = np.asarray(input_tensor, f)
    g = np.asarray(ln_gamma, f)
    be = np.asarray(ln_beta, f)
    Wq = np.asarray(Wq, f); Wk = np.asarray(Wk, f)
    Wv = np.asarray(Wv, f); Wo = np.asarray(Wo, f)
    wqt = np.ascontiguousarray((Wq * g[None, :]).T)        # [i, j]
    nrw = -wqt.sum(axis=0, keepdims=True)                  # [1, j]
    bq_eff = (np.asarray(bq, f) + be @ Wq.T).astype(f)
    per_core = {
        "wqt": wqt.astype(bf),
        "wkt": np.ascontiguousarray(Wk.T).astype(bf),
        "wvt": np.ascontiguousarray(Wv.T).astype(bf),
        "wot": np.ascontiguousarray(Wo.T * np.float32(1.0 / np.sqrt(HD))).astype(bf),
        "nrw": nrw.astype(bf),
    }
    biases = {"bq": bq_eff, "bk": np.asarray(bk, f),
              "bv": np.asarray(bv, f), "bo": np.asarray(bo, f)}
    has_bias = any(np.any(v) for v in biases.values())
    if has_bias:
        for nm, v in biases.items():
            per_core[nm] = v.reshape(1, HID).astype(bf)
    return np.ascontiguousarray(x.astype(bf)), per_core, has_bias


def kernel(**inputs) -> np.ndarray:
    x, per_core, has_bias = prep_inputs(**inputs)
    fn, in_names, out_names, out_avals = _get_runner(1, has_bias)

    concat_in = []
    for name in in_names:
        if name == "x":
            concat_in.append(x.reshape(B * S, HID))
        else:
            concat_in.append(np.concatenate([per_core[name]] * B, axis=0))
    concat_zeros = [
        np.zeros((B * av.shape[0], *av.shape[1:]), av.dtype) for av in out_avals
    ]
    out_arrs = fn(*concat_in, *concat_zeros)
    out = np.asarray(out_arrs[out_names.index("out")])
    return out.reshape(B, S, HID)


# revision 21
# speedup vs baseline: 1.1180x; 1.1180x over previous
"""Trainium2 Bass kernel for nn_MultiHeadAttention_84791244358011.

Linear (ELU feature-map) attention:
    x_norm = LayerNorm(x)                      # eps=1e-12
    q = x_norm @ Wq.T + bq ; k,v = x @ W.T + b # per-head [S, 64]
    eq/ek = l2norm(elu(q/k)) per token over head_dim
    kv = ek^T @ v per head [64, 64]; ctx = eq @ kv / 8
    out = ctx @ Wo.T + bo + x

Sharding: data-parallel over batch B=8 — one batch element per NeuronCore,
no collectives. All matmul operands are bf16 (PSUM accumulation stays f32).

Single pass over the 32 token tiles (vs. the old two-pass + DRAM spill):
  - x^T comes straight from DRAM through the DMA xbar transpose (one
    InstDmaTransposeAnt per tile) — no PE transposes, no PSUM copies.
  - LayerNorm is folded into the Q projection:
        q = rstd * (x @ wqt - mu x rw) + bq_eff,   rw_j = sum_i wqt[i,j]
    mu enters as a rank-1 matmul (mu row x -rw row); rstd enters as the
    per-partition *scale* operand of the ELU's two Relu activations.
    The mu row itself is a ones-column matmul against x^T, borrowed into
    the q0 PSUM bank before the q accumulation starts.
  - elu(x)+1 == relu(x) + exp(min(x,0)) exactly, so the ELU is
    relu(-x) -> exp(-.) on ACT, relu(x) on ACT, one add on Pool; the
    "-1" folds into Square's bias (sum-of-squares) and a tensor_scalar.
  - all rsqrt via exp(-0.5*ln(.)) on ACT: ln and exp live in the same
    activation table set (natural_log_exp_and_others) so there are no
    1.28us table swaps, and no DVE Newton chains.
  - the k-side l2 norm is applied to V instead of K (kv = sum ek x v is
    bilinear), saving a broadcast multiply from PSUM.
  - per-head kv state accumulates into a single PSUM bank: even heads at
    partitions 0-63, odd heads at 64-127 via tile_position=(0,64).
  - eq is written bf16 and xbar-transposed SBUF->SBUF into a persistent
    [128, 8, 4096] eqT buffer.
  - the per-head state is accumulated TRANSPOSED (vk_h = sum_s v x ek)
    so that pass 2 can fold the whole attention tail into one GEMM:
        out = eq @ M + x,   M = blockdiag(kv_h) @ wot
    M is built with 32 small matmuls once per iteration; this removes
    the per-chunk ctx matmuls and all 256 ctx^T PSUM->SBUF copies.
  - vk accumulation matmuls for tile t issue at the start of tile t+1
    so the PE never waits on the k-norm chain.

Biases are all zero for this problem's inputs; a with_bias variant adds
rank-1 (ones x bias) matmuls into each projection's PSUM group.
"""

import contextlib

import numpy as np

import concourse.bass as bass
import concourse.mybir as mybir
import concourse.tile as tile
from concourse import bacc

B, S, HID = 8, 4096, 1024
NH, HD = 16, 64
P = 128
NT = S // P            # 32 token tiles
NC = HID // P          # 8 feature chunks
CHUNK = 4              # token tiles per ctx chunk (512 tokens)
NCHUNKS = NT // CHUNK
LN_EPS = 1e-12

F32 = mybir.dt.float32
BF16 = mybir.dt.bfloat16
AF = mybir.ActivationFunctionType
OP = mybir.AluOpType
AX = mybir.AxisListType


def build_nc(with_bias=False, loop_n=1):
    nc = bacc.Bacc("TRN2", target_bir_lowering=False, enable_partition_id=False)

    x_d = nc.dram_tensor("x", [S, HID], BF16, kind="ExternalInput")
    wqt_d = nc.dram_tensor("wqt", [HID, HID], BF16, kind="ExternalInput")
    wkt_d = nc.dram_tensor("wkt", [HID, HID], BF16, kind="ExternalInput")
    wvt_d = nc.dram_tensor("wvt", [HID, HID], BF16, kind="ExternalInput")
    wot_d = nc.dram_tensor("wot", [HID, HID], BF16, kind="ExternalInput")
    nrw_d = nc.dram_tensor("nrw", [1, HID], BF16, kind="ExternalInput")
    bias_d = {}
    if with_bias:
        for nm in ("bq", "bk", "bv", "bo"):
            bias_d[nm] = nc.dram_tensor(nm, [1, HID], BF16, kind="ExternalInput")
    out_d = nc.dram_tensor("out", [S, HID], F32, kind="ExternalOutput")

    with tile.TileContext(nc) as tc, contextlib.ExitStack() as ctx:
        persist = ctx.enter_context(tc.tile_pool(name="persist", bufs=1))
        wpool = ctx.enter_context(tc.tile_pool(name="weights", bufs=1))

        ones_col = persist.tile([P, 1], BF16)
        nc.gpsimd.memset(ones_col, 1.0)
        eps_c = persist.tile([P, 1], F32)
        nc.gpsimd.memset(eps_c, LN_EPS)
        negone_c = persist.tile([P, 1], F32)
        nc.gpsimd.memset(negone_c, -1.0)
        kv_sb = persist.tile([P, NC * HD], BF16)      # packed kv state
        eqT = persist.tile([P, NC, S], BF16)          # transposed eq, full S
        nrw_sb = persist.tile([1, HID], BF16)
        nc.sync.dma_start(nrw_sb, nrw_d.ap())
        brow = {}
        if with_bias:
            ones_row = persist.tile([1, P], BF16)
            nc.gpsimd.memset(ones_row, 1.0)
            for nm, d in bias_d.items():
                t_ = persist.tile([1, HID], BF16, name=f"{nm}_row")
                nc.sync.dma_start(t_, d.ap())
                brow[nm] = t_

        _loop = tc.For_i(0, loop_n, 1) if loop_n > 1 else contextlib.nullcontext(0)
        with _loop:
            # ------------- pass 1: k/v/q, kv state, eqT -------------
            wk_sb = wpool.tile([P, NC, HID], BF16, tag="wA", name="wk_sb")
            nc.gpsimd.dma_start(wk_sb, wkt_d.ap().rearrange("(c p) j -> p c j", p=P))
            wv_sb = wpool.tile([P, NC, HID], BF16, tag="wB", name="wv_sb")
            nc.gpsimd.dma_start(wv_sb, wvt_d.ap().rearrange("(c p) j -> p c j", p=P))
            wq_sb = wpool.tile([P, NC, HID], BF16, tag="wC", name="wq_sb")
            nc.gpsimd.dma_start(wq_sb, wqt_d.ap().rearrange("(c p) j -> p c j", p=P))
            wo_sb = wpool.tile([P, NC, HID], BF16, tag="wD", name="wo_sb")
            nc.gpsimd.dma_start(wo_sb, wot_d.ap().rearrange("(c p) j -> p c j", p=P))

            with tc.tile_pool(name="sbufA", bufs=1) as sa, \
                 tc.tile_pool(name="psumA", bufs=1, space="PSUM") as pa:
                kv_ps = pa.tile([P, NC * HD], F32, tag="kv", name="kv_ps")
                # Explicitly zero the state bank: matmul start=True pend-zero
                # only covers the issuing matmul's own col-strip, so the odd
                # (partition 64-127) strip would otherwise accumulate forever
                # across kernel executions.
                nc.vector.memset(kv_ps, 0.0)
                pend_kv = []           # deferred kv matmuls (ek, vs) per tile

                def flush_kv(last):
                    if not pend_kv:
                        return
                    first, ek, vs = pend_kv.pop()
                    for a in range(NC):
                        nc.tensor.matmul(
                            kv_ps[0:HD, a * HD:(a + 1) * HD],
                            vs[:, 2 * a, :], ek[:, 2 * a, :],
                            start=False, stop=last,
                            tile_position=(0, 0), skip_group_check=True)
                        nc.tensor.matmul(
                            kv_ps[HD:P, a * HD:(a + 1) * HD],
                            vs[:, 2 * a + 1, :], ek[:, 2 * a + 1, :],
                            start=False, stop=last,
                            tile_position=(0, 64), skip_group_check=True)

                for t in range(NT):
                    tok = slice(t * P, (t + 1) * P)
                    x_t = sa.tile([P, HID], BF16, tag="x", bufs=3, name=f"x_{t}")
                    nc.scalar.dma_start(x_t, x_d.ap()[tok, :])
                    xT = sa.tile([P, NC, P], BF16, tag="xT", bufs=3,
                                 name=f"xT_{t}")
                    nc.sync.dma_start_transpose(xT, x_d.ap()[tok, :])

                    # LayerNorm stats; rstd = exp(-0.5*ln(var+eps))
                    stats = sa.tile([P, 2, 6], F32, tag="st", bufs=3,
                                    name=f"st_{t}")
                    xg = x_t[:].rearrange("p (g d) -> p g d", g=2)
                    for g in range(2):
                        nc.vector.bn_stats(stats[:, g, :], xg[:, g, :])
                    mv = sa.tile([P, 2], F32, tag="mv", bufs=3, name=f"mv_{t}")
                    nc.vector.bn_aggr(mv, stats)
                    lnv = sa.tile([P, 1], F32, tag="lnv", bufs=3, name=f"lnv_{t}")
                    nc.scalar.activation(lnv, mv[:, 1:2], AF.Ln, bias=eps_c[:, 0:1])
                    rstd = sa.tile([P, 1], F32, tag="rsd", bufs=3, name=f"rsd_{t}")
                    nc.scalar.activation(rstd, lnv, AF.Exp, scale=-0.5)
                    nrstd = sa.tile([P, 1], F32, tag="nrs", bufs=3, name=f"nrs_{t}")
                    nc.vector.tensor_scalar(nrstd, rstd, -1.0, None, OP.mult)

                    def proj(ps, w_sb, half, extras=()):
                        sl = slice(half * 512, (half + 1) * 512)
                        for c in range(NC):
                            nc.tensor.matmul(
                                ps, xT[:, c, :], w_sb[:, c, sl],
                                start=(c == 0),
                                stop=(c == NC - 1 and not extras),
                                skip_group_check=True)
                        for i, (lhs, rhs_row) in enumerate(extras):
                            nc.tensor.matmul(
                                ps, lhs, rhs_row[0:1, sl],
                                start=False, stop=(i == len(extras) - 1),
                                skip_group_check=True)

                    # kv accumulation for the previous tile (PE never waits)
                    flush_kv(last=False)

                    # ---- K ----
                    k_ps = [pa.tile([P, 512], F32, tag="kh", bufs=2,
                                    name=f"k_ps{t}_{h}") for h in range(2)]
                    kex = [(ones_row, brow["bk"])] if with_bias else []
                    w1k = sa.tile([P, HID], BF16, tag="w1k", bufs=3,
                                  name=f"w1k_{t}")
                    for half in range(2):
                        proj(k_ps[half], wk_sb, half, kex)
                        hs = slice(half * 512, (half + 1) * 512)
                        r = sa.tile([P, 512], BF16, tag="kr", bufs=3,
                                    name=f"kr_{t}_{half}")
                        nc.scalar.activation(r, k_ps[half], AF.Relu, scale=-1.0)
                        e = sa.tile([P, 512], BF16, tag="ke", bufs=3,
                                    name=f"ke_{t}_{half}")
                        nc.scalar.activation(e, r, AF.Exp, scale=-1.0)
                        m = sa.tile([P, 512], BF16, tag="km", bufs=3,
                                    name=f"km_{t}_{half}")
                        nc.scalar.activation(m, k_ps[half], AF.Relu)
                        nc.gpsimd.tensor_tensor(w1k[:, hs], m, e, OP.add)
                    sqk = sa.tile([P, NH, HD], BF16, tag="sqk", bufs=2,
                                  name=f"sqk_{t}")
                    nc.scalar.activation(
                        sqk[:].rearrange("p h d -> p (h d)"), w1k, AF.Square,
                        bias=negone_c[:, 0:1])
                    ssk = sa.tile([P, NH], F32, tag="ssk", bufs=3,
                                  name=f"ssk_{t}")
                    nc.vector.tensor_reduce(ssk, sqk, AX.X, OP.add)
                    lnk = sa.tile([P, NH], F32, tag="lnk", bufs=3,
                                  name=f"lnk_{t}")
                    nc.scalar.activation(lnk, ssk, AF.Ln)
                    rnk = sa.tile([P, NH], F32, tag="rnk", bufs=3,
                                  name=f"rnk_{t}")
                    nc.scalar.activation(rnk, lnk, AF.Exp, scale=-0.5)
                    ek = sa.tile([P, NH, HD], BF16, tag="ek", bufs=2,
                                 name=f"ek_{t}")
                    nc.vector.tensor_scalar(
                        ek[:].rearrange("p h d -> p (h d)"), w1k, 1.0, None,
                        OP.subtract)

                    # ---- V (k-norm folded in) ----
                    v_ps = [pa.tile([P, 512], F32, tag="vh", bufs=3,
                                    name=f"v_ps{t}_{h}") for h in range(2)]
                    vex = [(ones_row, brow["bv"])] if with_bias else []
                    vs = sa.tile([P, NH, HD], BF16, tag="vs", bufs=2,
                                 name=f"vs_{t}")
                    for half in range(2):
                        proj(v_ps[half], wv_sb, half, vex)
                        hh = slice(half * 8, (half + 1) * 8)
                        nc.vector.tensor_tensor(
                            vs[:, hh, :],
                            v_ps[half][:].rearrange("p (h d) -> p h d", d=HD),
                            rnk[:, hh, None].to_broadcast((P, 8, HD)),
                            OP.mult)

                    # mu row via ones-matmul, borrowed into the q0 psum bank
                    q_ps = [pa.tile([P, 512], F32, tag="qh", bufs=2,
                                    name=f"q_ps{t}_{h}") for h in range(2)]
                    for c in range(NC):
                        nc.tensor.matmul(q_ps[0][0:1, 0:P], ones_col,
                                         xT[:, c, :],
                                         start=(c == 0), stop=(c == NC - 1),
                                         skip_group_check=True)
                    mu_row = sa.tile([1, P], BF16, tag="mu", bufs=3,
                                     name=f"mu_{t}")
                    nc.vector.tensor_scalar(mu_row, q_ps[0][0:1, 0:P],
                                            1.0 / HID, None, OP.mult)

                    # ---- Q (LayerNorm folded in) ----
                    qex = [(mu_row, nrw_sb)]
                    if with_bias:
                        qex.append((ones_row, brow["bq"]))
                    w1q = sa.tile([P, HID], BF16, tag="w1q", bufs=3,
                                  name=f"w1q_{t}")
                    for half in range(2):
                        proj(q_ps[half], wq_sb, half, qex)
                        hs = slice(half * 512, (half + 1) * 512)
                        r = sa.tile([P, 512], BF16, tag="qr", bufs=3,
                                    name=f"qr_{t}_{half}")
                        nc.scalar.activation(r, q_ps[half], AF.Relu,
                                             scale=nrstd[:, 0:1])
                        e = sa.tile([P, 512], BF16, tag="qe", bufs=3,
                                    name=f"qe_{t}_{half}")
                        nc.scalar.activation(e, r, AF.Exp, scale=-1.0)
                        m = sa.tile([P, 512], BF16, tag="qm", bufs=3,
                                    name=f"qm_{t}_{half}")
                        nc.scalar.activation(m, q_ps[half], AF.Relu,
                                             scale=rstd[:, 0:1])
                        nc.gpsimd.tensor_tensor(w1q[:, hs], m, e, OP.add)
                    sqq = sa.tile([P, NH, HD], BF16, tag="sqq", bufs=2,
                                  name=f"sqq_{t}")
                    nc.scalar.activation(
                        sqq[:].rearrange("p h d -> p (h d)"), w1q, AF.Square,
                        bias=negone_c[:, 0:1])
                    ssq = sa.tile([P, NH], F32, tag="ssq", bufs=3,
                                  name=f"ssq_{t}")
                    nc.vector.tensor_reduce(ssq, sqq, AX.X, OP.add)
                    lnq = sa.tile([P, NH], F32, tag="lnq", bufs=3,
                                  name=f"lnq_{t}")
                    nc.scalar.activation(lnq, ssq, AF.Ln)
                    rnq = sa.tile([P, NH], BF16, tag="rnq", bufs=3,
                                  name=f"rnq_{t}")
                    nc.scalar.activation(rnq, lnq, AF.Exp, scale=-0.5)
                    eqp = sa.tile([P, NH, HD], BF16, tag="eqp", bufs=2,
                                  name=f"eqp_{t}")
                    nc.vector.tensor_scalar(
                        eqp[:].rearrange("p h d -> p (h d)"), w1q, 1.0, None,
                        OP.subtract)
                    eq = sa.tile([P, NH, HD], BF16, tag="eq", bufs=2,
                                 name=f"eq_{t}")
                    nc.vector.tensor_tensor(
                        eq, eqp, rnq[:, :, None].to_broadcast((P, NH, HD)),
                        OP.mult)
                    nc.sync.dma_start_transpose(
                        eqT[:, :, tok], eq[:].rearrange("p h d -> p (h d)"))

                    pend_kv.append((t == 0, ek, vs))

                flush_kv(last=True)
                nc.vector.tensor_copy(kv_sb, kv_ps)

            # ------------- pass 2: out = eq @ (blockdiag(kv) @ wot) + x --
            with tc.tile_pool(name="sbufB", bufs=1) as sb, \
                 tc.tile_pool(name="psumB", bufs=1, space="PSUM") as pb:
                # M[c-chunk][p, j]: rows = (head, d) feature index c*128+p.
                # vk_sb holds per-head vk[e, d]: head 2a at partitions 0-63,
                # col block a; head 2a+1 at partitions 64-127, col block a.
                m_sb = sb.tile([P, NC, HID], BF16, tag="m", bufs=1, name="m_sb")
                for c in range(NC):
                    for half in range(2):
                        hs = slice(half * 512, (half + 1) * 512)
                        m_pse = pb.tile([HD, 512], F32, tag="mpse", bufs=2,
                                        name=f"m_pse{c}_{half}")
                        m_pso = pb.tile([HD, 512], F32, tag="mpso", bufs=2,
                                        name=f"m_pso{c}_{half}")
                        nc.tensor.matmul(
                            m_pse, kv_sb[0:HD, c * HD:(c + 1) * HD],
                            wo_sb[0:HD, c, hs], start=True, stop=True)
                        nc.tensor.matmul(
                            m_pso, kv_sb[HD:P, c * HD:(c + 1) * HD],
                            wo_sb[HD:P, c, hs], start=True, stop=True)
                        nc.scalar.copy(m_sb[0:HD, c, hs], m_pse)
                        nc.scalar.copy(m_sb[HD:P, c, hs], m_pso)

                for t in range(NT):
                    tok = slice(t * P, (t + 1) * P)
                    x2 = sb.tile([P, HID], BF16, tag="x2", bufs=2,
                                 name=f"x2_{t}")
                    nc.scalar.dma_start(x2, x_d.ap()[tok, :])
                    out_sb = sb.tile([P, HID], F32, tag="osb", bufs=2,
                                     name=f"out_{t}")
                    for half in range(2):
                        hs = slice(half * 512, (half + 1) * 512)
                        o_ps = pb.tile([P, 512], F32, tag="oh", bufs=3,
                                       name=f"o_ps{t}_{half}")
                        for c in range(NC):
                            nc.tensor.matmul(
                                o_ps, eqT[:, c, tok], m_sb[:, c, hs],
                                start=(c == 0),
                                stop=(c == NC - 1 and not with_bias),
                                skip_group_check=True)
                        if with_bias:
                            nc.tensor.matmul(
                                o_ps, ones_row, brow["bo"][0:1, hs],
                                start=False, stop=True,
                                skip_group_check=True)
                        nc.vector.tensor_tensor(
                            out_sb[:, hs], o_ps, x2[:, hs], OP.add)
                    nc.gpsimd.dma_start(out_d.ap()[tok, :], out_sb)

    nc.compile()
    _force_single_act_table(nc)
    return nc


def _force_single_act_table(nc):
    """Retarget all activation-table loads to natural_log_exp_and_others
    (which contains every function this kernel uses: ln, exp, relu, square,
    copy) and drop the now-redundant reloads. The insertion pass picks the
    first table containing each function, which thrashes exp<->ln at 1.28us
    per reload, ~6x per token tile."""
    from concourse.hw_specs import get_activation_tables

    names = list(get_activation_tables(nc.m.arch).keys())
    target = names.index("natural_log_exp_and_others")
    kept_one = False
    for b in nc.main_func.blocks:
        keep = []
        for i in b.instructions:
            if isinstance(i, mybir.InstLoadActFuncSet):
                si = getattr(i, "sync_info", None)
                has_sync = si is not None and (
                    len(si.on_wait) > 0 or len(si.on_update) > 0)
                i.act_func_set_id = target
                if not kept_one or has_sync:
                    kept_one = True
                    keep.append(i)
            else:
                keep.append(i)
        b.instructions[:] = keep


_RUNNER = {}


def _get_runner(loop_n=1, with_bias=False):
    key = (loop_n, with_bias)
    if key in _RUNNER:
        return _RUNNER[key]

    import jax
    from jax.sharding import Mesh, PartitionSpec
    from jax.experimental.shard_map import shard_map
    from concourse.bass2jax import _bass_exec_p, install_neuronx_cc_hook

    install_neuronx_cc_hook()
    nc = build_nc(with_bias=with_bias, loop_n=loop_n)

    in_names = []
    out_names = []
    out_avals = []
    for alloc in nc.m.functions[0].allocations:
        if not isinstance(alloc, mybir.MemoryLocationSet):
            continue
        name = alloc.memorylocations[0].name
        if alloc.kind == "ExternalInput":
            in_names.append(name)
        elif alloc.kind == "ExternalOutput":
            out_names.append(name)
            out_avals.append(
                jax.core.ShapedArray(tuple(alloc.tensor_shape),
                                     mybir.dt.np(alloc.dtype)))
    n_params = len(in_names)
    all_in_names = in_names + out_names

    def _body(*args):
        outs = _bass_exec_p.bind(
            *args,
            out_avals=tuple(out_avals),
            in_names=tuple(all_in_names),
            out_names=tuple(out_names),
            lowering_input_output_aliases=(),
            sim_require_finite=True,
            sim_require_nnan=True,
            nc=nc,
        )
        return tuple(outs)

    devices = jax.devices()[:B]
    mesh = Mesh(np.asarray(devices), ("core",))
    n_outs = len(out_names)
    fn = jax.jit(
        shard_map(
            _body, mesh=mesh,
            in_specs=(PartitionSpec("core"),) * (n_params + n_outs),
            out_specs=(PartitionSpec("core"),) * n_outs,
            check_rep=False,
        ),
        keep_unused=True,
    )
    _RUNNER[key] = (fn, in_names, out_names, out_avals)
    return _RUNNER[key]


def prep_inputs(input_tensor, attention_mask, ln_gamma, ln_beta,
                Wq, bq, Wk, bk, Wv, bv, Wo, bo):
    """Host-side static prep: transpose weights, fold gamma/beta/scale."""
    import ml_dtypes
    bf = ml_dtypes.bfloat16
    f = np.float32
    x = np.asarray(input_tensor, f)
    g = np.asarray(ln_gamma, f)
    be = np.asarray(ln_beta, f)
    Wq = np.asarray(Wq, f); Wk = np.asarray(Wk, f)
    Wv = np.asarray(Wv, f); Wo = np.asarray(Wo, f)
    wqt = np.ascontiguousarray((Wq * g[None, :]).T)        # [i, j]
    nrw = -wqt.sum(axis=0, keepdims=True)                  # [1, j]
    bq_eff = (np.asarray(bq, f) + be @ Wq.T).astype(f)
    per_core = {
        "wqt": wqt.astype(bf),
        "wkt": np.ascontiguousarray(Wk.T).astype(bf),
        "wvt": np.ascontiguousarray(Wv.T).astype(bf),
        "wot": np.ascontiguousarray(Wo.T * np.float32(1.0 / np.sqrt(HD))).astype(bf),
        "nrw": nrw.astype(bf),
    }
    biases = {"bq": bq_eff, "bk": np.asarray(bk, f),
              "bv": np.asarray(bv, f), "bo": np.asarray(bo, f)}
    has_bias = any(np.any(v) for v in biases.values())
    if has_bias:
        for nm, v in biases.items():
            per_core[nm] = v.reshape(1, HID).astype(bf)
    return np.ascontiguousarray(x.astype(bf)), per_core, has_bias


def kernel(**inputs) -> np.ndarray:
    x, per_core, has_bias = prep_inputs(**inputs)
    fn, in_names, out_names, out_avals = _get_runner(1, has_bias)

    concat_in = []
    for name in in_names:
        if name == "x":
            concat_in.append(x.reshape(B * S, HID))
        else:
            concat_in.append(np.concatenate([per_core[name]] * B, axis=0))
    concat_zeros = [
        np.zeros((B * av.shape[0], *av.shape[1:]), av.dtype) for av in out_avals
    ]
    out_arrs = fn(*concat_in, *concat_zeros)
    out = np.asarray(out_arrs[out_names.index("out")])
    return out.reshape(B, S, HID)


# revision 22
# speedup vs baseline: 1.1278x; 1.0087x over previous
"""Trainium2 Bass kernel for nn_MultiHeadAttention_84791244358011.

Linear (ELU feature-map) attention:
    x_norm = LayerNorm(x)                      # eps=1e-12
    q = x_norm @ Wq.T + bq ; k,v = x @ W.T + b # per-head [S, 64]
    eq/ek = l2norm(elu(q/k)) per token over head_dim
    kv = ek^T @ v per head [64, 64]; ctx = eq @ kv / 8
    out = ctx @ Wo.T + bo + x

Sharding: data-parallel over batch B=8 — one batch element per NeuronCore,
no collectives. All matmul operands are bf16 (PSUM accumulation stays f32).

Single pass over the 32 token tiles (vs. the old two-pass + DRAM spill):
  - x^T comes straight from DRAM through the DMA xbar transpose (one
    InstDmaTransposeAnt per tile) — no PE transposes, no PSUM copies.
  - LayerNorm is folded into the Q projection:
        q = rstd * (x @ wqt - mu x rw) + bq_eff,   rw_j = sum_i wqt[i,j]
    mu enters as a rank-1 matmul (mu row x -rw row); rstd enters as the
    per-partition *scale* operand of the ELU's two Relu activations.
    The mu row itself is a ones-column matmul against x^T, borrowed into
    the q0 PSUM bank before the q accumulation starts.
  - elu(x)+1 == relu(x) + exp(min(x,0)) exactly, so the ELU is
    relu(-x) -> exp(-.) on ACT, relu(x) on ACT, one add on Pool; the
    "-1" folds into Square's bias (sum-of-squares) and a tensor_scalar.
  - all rsqrt via exp(-0.5*ln(.)) on ACT: ln and exp live in the same
    activation table set (natural_log_exp_and_others) so there are no
    1.28us table swaps, and no DVE Newton chains.
  - the k-side l2 norm is applied to V instead of K (kv = sum ek x v is
    bilinear), saving a broadcast multiply from PSUM.
  - per-head kv state accumulates into a single PSUM bank: even heads at
    partitions 0-63, odd heads at 64-127 via tile_position=(0,64).
  - eq is written bf16 and xbar-transposed SBUF->SBUF into a persistent
    [128, 8, 4096] eqT buffer.
  - the per-head state is accumulated TRANSPOSED (vk_h = sum_s v x ek)
    so that pass 2 can fold the whole attention tail into one GEMM:
        out = eq @ M + x,   M = blockdiag(kv_h) @ wot
    M is built with 32 small matmuls once per iteration; this removes
    the per-chunk ctx matmuls and all 256 ctx^T PSUM->SBUF copies.
  - vk accumulation matmuls for tile t issue at the start of tile t+1
    so the PE never waits on the k-norm chain.

Biases are all zero for this problem's inputs; a with_bias variant adds
rank-1 (ones x bias) matmuls into each projection's PSUM group.
"""

import contextlib

import numpy as np

import concourse.bass as bass
import concourse.mybir as mybir
import concourse.tile as tile
from concourse import bacc
from concourse.masks import make_identity

B, S, HID = 8, 4096, 1024
NH, HD = 16, 64
P = 128
NT = S // P            # 32 token tiles
NC = HID // P          # 8 feature chunks
CHUNK = 4              # token tiles per ctx chunk (512 tokens)
NCHUNKS = NT // CHUNK
LN_EPS = 1e-12

F32 = mybir.dt.float32
BF16 = mybir.dt.bfloat16
AF = mybir.ActivationFunctionType
OP = mybir.AluOpType
AX = mybir.AxisListType


def build_nc(with_bias=False, loop_n=1):
    nc = bacc.Bacc("TRN2", target_bir_lowering=False, enable_partition_id=False)

    x_d = nc.dram_tensor("x", [S, HID], BF16, kind="ExternalInput")
    wqt_d = nc.dram_tensor("wqt", [HID, HID], BF16, kind="ExternalInput")
    wkt_d = nc.dram_tensor("wkt", [HID, HID], BF16, kind="ExternalInput")
    wvt_d = nc.dram_tensor("wvt", [HID, HID], BF16, kind="ExternalInput")
    wot_d = nc.dram_tensor("wot", [HID, HID], BF16, kind="ExternalInput")
    nrw_d = nc.dram_tensor("nrw", [1, HID], BF16, kind="ExternalInput")
    bias_d = {}
    if with_bias:
        for nm in ("bq", "bk", "bv", "bo"):
            bias_d[nm] = nc.dram_tensor(nm, [1, HID], BF16, kind="ExternalInput")
    out_d = nc.dram_tensor("out", [S, HID], F32, kind="ExternalOutput")

    with tile.TileContext(nc) as tc, contextlib.ExitStack() as ctx:
        persist = ctx.enter_context(tc.tile_pool(name="persist", bufs=1))
        wpool = ctx.enter_context(tc.tile_pool(name="weights", bufs=1))

        ident = persist.tile([P, P], F32)
        make_identity(nc, ident)
        eps_c = persist.tile([P, 1], F32)
        nc.gpsimd.memset(eps_c, LN_EPS)
        negone_c = persist.tile([P, 1], F32)
        nc.gpsimd.memset(negone_c, -1.0)
        kv_sb = persist.tile([P, NC, P], BF16)        # packed vk state
        eqT = persist.tile([P, NC, S], BF16)          # transposed eq, full S
        nrw_sb = persist.tile([1, HID], BF16)
        nc.sync.dma_start(nrw_sb, nrw_d.ap())
        brow = {}
        if with_bias:
            ones_row = persist.tile([1, P], BF16)
            nc.gpsimd.memset(ones_row, 1.0)
            for nm, d in bias_d.items():
                t_ = persist.tile([1, HID], BF16, name=f"{nm}_row")
                nc.sync.dma_start(t_, d.ap())
                brow[nm] = t_

        _loop = tc.For_i(0, loop_n, 1) if loop_n > 1 else contextlib.nullcontext(0)
        with _loop:
            # ------------- pass 1: k/v/q, kv state, eqT -------------
            wk_sb = wpool.tile([P, NC, HID], BF16, tag="wA", name="wk_sb")
            nc.gpsimd.dma_start(wk_sb, wkt_d.ap().rearrange("(c p) j -> p c j", p=P))
            wv_sb = wpool.tile([P, NC, HID], BF16, tag="wB", name="wv_sb")
            nc.gpsimd.dma_start(wv_sb, wvt_d.ap().rearrange("(c p) j -> p c j", p=P))
            wq_sb = wpool.tile([P, NC, HID], BF16, tag="wC", name="wq_sb")
            nc.gpsimd.dma_start(wq_sb, wqt_d.ap().rearrange("(c p) j -> p c j", p=P))
            wo_sb = wpool.tile([P, NC, HID], BF16, tag="wD", name="wo_sb")
            nc.gpsimd.dma_start(wo_sb, wot_d.ap().rearrange("(c p) j -> p c j", p=P))

            with tc.tile_pool(name="sbufA", bufs=1) as sa, \
                 tc.tile_pool(name="psumA", bufs=1, space="PSUM") as pa:
                # vk state per head pair a: kv_ps[:, a, :] = [vs_2a|vs_2a+1]^T
                # @ [ek_2a|ek_2a+1]; diagonal 64x64 blocks are vk_2a / vk_2a+1,
                # off-diagonal cross-head blocks are never read. Explicitly
                # zeroed (start=True pend-zero covers only the issuing
                # matmul's col-strip, and PSUM persists across executions).
                kv_ps = pa.tile([P, NC, P], F32, tag="kv", name="kv_ps")
                nc.vector.memset(kv_ps, 0.0)
                pend_kv = []           # deferred kv matmuls (ek, vs) per tile

                def flush_kv(last):
                    if not pend_kv:
                        return
                    first, ek, vs = pend_kv.pop()
                    ekf = ek[:].rearrange("p h d -> p (h d)")
                    vsf = vs[:].rearrange("p h d -> p (h d)")
                    for a in range(NC):
                        nc.tensor.matmul(
                            kv_ps[:, a, :],
                            vsf[:, a * P:(a + 1) * P],
                            ekf[:, a * P:(a + 1) * P],
                            start=False, stop=last, skip_group_check=True)

                for t in range(NT):
                    tok = slice(t * P, (t + 1) * P)
                    x_t = sa.tile([P, HID], BF16, tag="x", bufs=3, name=f"x_{t}")
                    nc.scalar.dma_start(x_t, x_d.ap()[tok, :])
                    xT = sa.tile([P, NC, P], BF16, tag="xT", bufs=3,
                                 name=f"xT_{t}")
                    nc.sync.dma_start_transpose(xT, x_d.ap()[tok, :])

                    # LayerNorm stats; rstd = exp(-0.5*ln(var+eps))
                    stats = sa.tile([P, 2, 6], F32, tag="st", bufs=3,
                                    name=f"st_{t}")
                    xg = x_t[:].rearrange("p (g d) -> p g d", g=2)
                    for g in range(2):
                        nc.vector.bn_stats(stats[:, g, :], xg[:, g, :])
                    mv = sa.tile([P, 2], F32, tag="mv", bufs=3, name=f"mv_{t}")
                    nc.vector.bn_aggr(mv, stats)
                    lnv = sa.tile([P, 1], F32, tag="lnv", bufs=3, name=f"lnv_{t}")
                    nc.scalar.activation(lnv, mv[:, 1:2], AF.Ln, bias=eps_c[:, 0:1])
                    rstd = sa.tile([P, 1], F32, tag="rsd", bufs=3, name=f"rsd_{t}")
                    nc.scalar.activation(rstd, lnv, AF.Exp, scale=-0.5)
                    nrstd = sa.tile([P, 1], F32, tag="nrs", bufs=3, name=f"nrs_{t}")
                    nc.vector.tensor_scalar(nrstd, rstd, -1.0, None, OP.mult)

                    def proj(ps, w_sb, half, extras=()):
                        sl = slice(half * 512, (half + 1) * 512)
                        for c in range(NC):
                            nc.tensor.matmul(
                                ps, xT[:, c, :], w_sb[:, c, sl],
                                start=(c == 0),
                                stop=(c == NC - 1 and not extras),
                                skip_group_check=True)
                        for i, (lhs, rhs_row) in enumerate(extras):
                            nc.tensor.matmul(
                                ps, lhs, rhs_row[0:1, sl],
                                start=False, stop=(i == len(extras) - 1),
                                skip_group_check=True)

                    # kv accumulation for the previous tile (PE never waits)
                    flush_kv(last=False)

                    # ---- K ----
                    k_ps = [pa.tile([P, 512], F32, tag="kh", bufs=2,
                                    name=f"k_ps{t}_{h}") for h in range(2)]
                    kex = [(ones_row, brow["bk"])] if with_bias else []
                    w1k = sa.tile([P, HID], BF16, tag="w1k", bufs=3,
                                  name=f"w1k_{t}")
                    for half in range(2):
                        proj(k_ps[half], wk_sb, half, kex)
                        hs = slice(half * 512, (half + 1) * 512)
                        r = sa.tile([P, 512], BF16, tag="kr", bufs=3,
                                    name=f"kr_{t}_{half}")
                        nc.scalar.activation(r, k_ps[half], AF.Relu, scale=-1.0)
                        e = sa.tile([P, 512], BF16, tag="ke", bufs=3,
                                    name=f"ke_{t}_{half}")
                        nc.scalar.activation(e, r, AF.Exp, scale=-1.0)
                        m = sa.tile([P, 512], BF16, tag="km", bufs=3,
                                    name=f"km_{t}_{half}")
                        nc.scalar.activation(m, k_ps[half], AF.Relu)
                        nc.gpsimd.tensor_tensor(w1k[:, hs], m, e, OP.add)
                    sqk = sa.tile([P, NH, HD], BF16, tag="sqk", bufs=2,
                                  name=f"sqk_{t}")
                    nc.scalar.activation(
                        sqk[:].rearrange("p h d -> p (h d)"), w1k, AF.Square,
                        bias=negone_c[:, 0:1])
                    ssk = sa.tile([P, NH], F32, tag="ssk", bufs=3,
                                  name=f"ssk_{t}")
                    nc.vector.tensor_reduce(ssk, sqk, AX.X, OP.add)
                    lnk = sa.tile([P, NH], F32, tag="lnk", bufs=3,
                                  name=f"lnk_{t}")
                    nc.scalar.activation(lnk, ssk, AF.Ln)
                    rnk = sa.tile([P, NH], F32, tag="rnk", bufs=3,
                                  name=f"rnk_{t}")
                    nc.scalar.activation(rnk, lnk, AF.Exp, scale=-0.5)
                    ek = sa.tile([P, NH, HD], BF16, tag="ek", bufs=2,
                                 name=f"ek_{t}")
                    nc.vector.tensor_scalar(
                        ek[:].rearrange("p h d -> p (h d)"), w1k, 1.0, None,
                        OP.subtract)

                    # ---- V (k-norm folded in) ----
                    v_ps = [pa.tile([P, 512], F32, tag="vh", bufs=2,
                                    name=f"v_ps{t}_{h}") for h in range(2)]
                    vex = [(ones_row, brow["bv"])] if with_bias else []
                    vs = sa.tile([P, NH, HD], BF16, tag="vs", bufs=2,
                                 name=f"vs_{t}")
                    for half in range(2):
                        proj(v_ps[half], wv_sb, half, vex)
                        hh = slice(half * 8, (half + 1) * 8)
                        nc.vector.tensor_tensor(
                            vs[:, hh, :],
                            v_ps[half][:].rearrange("p (h d) -> p h d", d=HD),
                            rnk[:, hh, None].to_broadcast((P, 8, HD)),
                            OP.mult)

                    # mu row via one PE transpose of [mu, var], borrowed
                    # into the q0 psum bank before the q accumulation
                    q_ps = [pa.tile([P, 512], F32, tag="qh", bufs=2,
                                    name=f"q_ps{t}_{h}") for h in range(2)]
                    nc.tensor.transpose(q_ps[0][0:2, 0:P], mv, ident)
                    mu_row = sa.tile([1, P], BF16, tag="mu", bufs=3,
                                     name=f"mu_{t}")
                    nc.vector.tensor_scalar(mu_row, q_ps[0][0:1, 0:P],
                                            1.0, None, OP.mult)

                    # ---- Q (LayerNorm folded in) ----
                    qex = [(mu_row, nrw_sb)]
                    if with_bias:
                        qex.append((ones_row, brow["bq"]))
                    w1q = sa.tile([P, HID], BF16, tag="w1q", bufs=3,
                                  name=f"w1q_{t}")
                    for half in range(2):
                        proj(q_ps[half], wq_sb, half, qex)
                        hs = slice(half * 512, (half + 1) * 512)
                        r = sa.tile([P, 512], BF16, tag="qr", bufs=3,
                                    name=f"qr_{t}_{half}")
                        nc.scalar.activation(r, q_ps[half], AF.Relu,
                                             scale=nrstd[:, 0:1])
                        e = sa.tile([P, 512], BF16, tag="qe", bufs=3,
                                    name=f"qe_{t}_{half}")
                        nc.scalar.activation(e, r, AF.Exp, scale=-1.0)
                        m = sa.tile([P, 512], BF16, tag="qm", bufs=3,
                                    name=f"qm_{t}_{half}")
                        nc.scalar.activation(m, q_ps[half], AF.Relu,
                                             scale=rstd[:, 0:1])
                        nc.gpsimd.tensor_tensor(w1q[:, hs], m, e, OP.add)
                    sqq = sa.tile([P, NH, HD], BF16, tag="sqq", bufs=2,
                                  name=f"sqq_{t}")
                    nc.scalar.activation(
                        sqq[:].rearrange("p h d -> p (h d)"), w1q, AF.Square,
                        bias=negone_c[:, 0:1])
                    ssq = sa.tile([P, NH], F32, tag="ssq", bufs=3,
                                  name=f"ssq_{t}")
                    nc.vector.tensor_reduce(ssq, sqq, AX.X, OP.add)
                    lnq = sa.tile([P, NH], F32, tag="lnq", bufs=3,
                                  name=f"lnq_{t}")
                    nc.scalar.activation(lnq, ssq, AF.Ln)
                    rnq = sa.tile([P, NH], BF16, tag="rnq", bufs=3,
                                  name=f"rnq_{t}")
                    nc.scalar.activation(rnq, lnq, AF.Exp, scale=-0.5)
                    eqp = sa.tile([P, NH, HD], BF16, tag="eqp", bufs=2,
                                  name=f"eqp_{t}")
                    nc.vector.tensor_scalar(
                        eqp[:].rearrange("p h d -> p (h d)"), w1q, 1.0, None,
                        OP.subtract)
                    eq = sa.tile([P, NH, HD], BF16, tag="eq", bufs=2,
                                 name=f"eq_{t}")
                    nc.vector.tensor_tensor(
                        eq, eqp, rnq[:, :, None].to_broadcast((P, NH, HD)),
                        OP.mult)
                    nc.sync.dma_start_transpose(
                        eqT[:, :, tok], eq[:].rearrange("p h d -> p (h d)"))

                    pend_kv.append((t == 0, ek, vs))

                flush_kv(last=True)
                nc.vector.tensor_copy(kv_sb, kv_ps)

            # ------------- pass 2: out = eq @ (blockdiag(kv) @ wot) + x --
            with tc.tile_pool(name="sbufB", bufs=1) as sb, \
                 tc.tile_pool(name="psumB", bufs=1, space="PSUM") as pb:
                # M[c-chunk][p, j]: rows = (head, d) feature index c*128+p.
                # vk_sb holds per-head vk[e, d]: head 2a at partitions 0-63,
                # col block a; head 2a+1 at partitions 64-127, col block a.
                m_sb = sb.tile([P, NC, HID], BF16, tag="m", bufs=1, name="m_sb")
                for c in range(NC):
                    for half in range(2):
                        hs = slice(half * 512, (half + 1) * 512)
                        m_pse = pb.tile([HD, 512], F32, tag="mpse", bufs=2,
                                        name=f"m_pse{c}_{half}")
                        m_pso = pb.tile([HD, 512], F32, tag="mpso", bufs=2,
                                        name=f"m_pso{c}_{half}")
                        nc.tensor.matmul(
                            m_pse, kv_sb[0:HD, c, 0:HD],
                            wo_sb[0:HD, c, hs], start=True, stop=True)
                        nc.tensor.matmul(
                            m_pso, kv_sb[HD:P, c, HD:P],
                            wo_sb[HD:P, c, hs], start=True, stop=True)
                        nc.scalar.copy(m_sb[0:HD, c, hs], m_pse)
                        nc.scalar.copy(m_sb[HD:P, c, hs], m_pso)

                for t in range(NT):
                    tok = slice(t * P, (t + 1) * P)
                    x2 = sb.tile([P, HID], BF16, tag="x2", bufs=2,
                                 name=f"x2_{t}")
                    nc.scalar.dma_start(x2, x_d.ap()[tok, :])
                    out_sb = sb.tile([P, HID], F32, tag="osb", bufs=2,
                                     name=f"out_{t}")
                    for half in range(2):
                        hs = slice(half * 512, (half + 1) * 512)
                        o_ps = pb.tile([P, 512], F32, tag="oh", bufs=3,
                                       name=f"o_ps{t}_{half}")
                        for c in range(NC):
                            nc.tensor.matmul(
                                o_ps, eqT[:, c, tok], m_sb[:, c, hs],
                                start=(c == 0),
                                stop=(c == NC - 1 and not with_bias),
                                skip_group_check=True)
                        if with_bias:
                            nc.tensor.matmul(
                                o_ps, ones_row, brow["bo"][0:1, hs],
                                start=False, stop=True,
                                skip_group_check=True)
                        nc.vector.tensor_tensor(
                            out_sb[:, hs], o_ps, x2[:, hs], OP.add)
                    nc.gpsimd.dma_start(out_d.ap()[tok, :], out_sb)

    nc.compile()
    _force_single_act_table(nc)
    return nc


def _force_single_act_table(nc):
    """Retarget all activation-table loads to natural_log_exp_and_others
    (which contains every function this kernel uses: ln, exp, relu, square,
    copy) and drop the now-redundant reloads. The insertion pass picks the
    first table containing each function, which thrashes exp<->ln at 1.28us
    per reload, ~6x per token tile."""
    from concourse.hw_specs import get_activation_tables

    names = list(get_activation_tables(nc.m.arch).keys())
    target = names.index("natural_log_exp_and_others")
    kept_one = False
    for b in nc.main_func.blocks:
        keep = []
        for i in b.instructions:
            if isinstance(i, mybir.InstLoadActFuncSet):
                si = getattr(i, "sync_info", None)
                has_sync = si is not None and (
                    len(si.on_wait) > 0 or len(si.on_update) > 0)
                i.act_func_set_id = target
                if not kept_one or has_sync:
                    kept_one = True
                    keep.append(i)
            else:
                keep.append(i)
        b.instructions[:] = keep


_RUNNER = {}


def _get_runner(loop_n=1, with_bias=False):
    key = (loop_n, with_bias)
    if key in _RUNNER:
        return _RUNNER[key]

    import jax
    from jax.sharding import Mesh, PartitionSpec
    from jax.experimental.shard_map import shard_map
    from concourse.bass2jax import _bass_exec_p, install_neuronx_cc_hook

    install_neuronx_cc_hook()
    nc = build_nc(with_bias=with_bias, loop_n=loop_n)

    in_names = []
    out_names = []
    out_avals = []
    for alloc in nc.m.functions[0].allocations:
        if not isinstance(alloc, mybir.MemoryLocationSet):
            continue
        name = alloc.memorylocations[0].name
        if alloc.kind == "ExternalInput":
            in_names.append(name)
        elif alloc.kind == "ExternalOutput":
            out_names.append(name)
            out_avals.append(
                jax.core.ShapedArray(tuple(alloc.tensor_shape),
                                     mybir.dt.np(alloc.dtype)))
    n_params = len(in_names)
    all_in_names = in_names + out_names

    def _body(*args):
        outs = _bass_exec_p.bind(
            *args,
            out_avals=tuple(out_avals),
            in_names=tuple(all_in_names),
            out_names=tuple(out_names),
            lowering_input_output_aliases=(),
            sim_require_finite=True,
            sim_require_nnan=True,
            nc=nc,
        )
        return tuple(outs)

    devices = jax.devices()[:B]
    mesh = Mesh(np.asarray(devices), ("core",))
    n_outs = len(out_names)
    fn = jax.jit(
        shard_map(
            _body, mesh=mesh,
            in_specs=(PartitionSpec("core"),) * (n_params + n_outs),
            out_specs=(PartitionSpec("core"),) * n_outs,
            check_rep=False,
        ),
        keep_unused=True,
    )
    _RUNNER[key] = (fn, in_names, out_names, out_avals)
    return _RUNNER[key]


def prep_inputs(input_tensor, attention_mask, ln_gamma, ln_beta,
                Wq, bq, Wk, bk, Wv, bv, Wo, bo):
    """Host-side static prep: transpose weights, fold gamma/beta/scale."""
    import ml_dtypes
    bf = ml_dtypes.bfloat16
    f = np.float32
    x = np.asarray(input_tensor, f)
    g = np.asarray(ln_gamma, f)
    be = np.asarray(ln_beta, f)
    Wq = np.asarray(Wq, f); Wk = np.asarray(Wk, f)
    Wv = np.asarray(Wv, f); Wo = np.asarray(Wo, f)
    wqt = np.ascontiguousarray((Wq * g[None, :]).T)        # [i, j]
    nrw = -wqt.sum(axis=0, keepdims=True)                  # [1, j]
    bq_eff = (np.asarray(bq, f) + be @ Wq.T).astype(f)
    per_core = {
        "wqt": wqt.astype(bf),
        "wkt": np.ascontiguousarray(Wk.T).astype(bf),
        "wvt": np.ascontiguousarray(Wv.T).astype(bf),
        "wot": np.ascontiguousarray(Wo.T * np.float32(1.0 / np.sqrt(HD))).astype(bf),
        "nrw": nrw.astype(bf),
    }
    biases = {"bq": bq_eff, "bk": np.asarray(bk, f),
              "bv": np.asarray(bv, f), "bo": np.asarray(bo, f)}
    has_bias = any(np.any(v) for v in biases.values())
    if has_bias:
        for nm, v in biases.items():
            per_core[nm] = v.reshape(1, HID).astype(bf)
    return np.ascontiguousarray(x.astype(bf)), per_core, has_bias


def kernel(**inputs) -> np.ndarray:
    x, per_core, has_bias = prep_inputs(**inputs)
    fn, in_names, out_names, out_avals = _get_runner(1, has_bias)

    concat_in = []
    for name in in_names:
        if name == "x":
            concat_in.append(x.reshape(B * S, HID))
        else:
            concat_in.append(np.concatenate([per_core[name]] * B, axis=0))
    concat_zeros = [
        np.zeros((B * av.shape[0], *av.shape[1:]), av.dtype) for av in out_avals
    ]
    out_arrs = fn(*concat_in, *concat_zeros)
    out = np.asarray(out_arrs[out_names.index("out")])
    return out.reshape(B, S, HID)


# revision 23
# speedup vs baseline: 1.6433x; 1.4571x over previous
"""Trainium2 Bass kernel for nn_MultiHeadAttention_84791244358011.

Linear (ELU feature-map) attention:
    x_norm = LayerNorm(x)                      # eps=1e-12
    q = x_norm @ Wq.T + bq ; k,v = x @ W.T + b # per-head [S, 64]
    eq/ek = l2norm(elu(q/k)) per token over head_dim
    kv = ek^T @ v per head [64, 64]; ctx = eq @ kv / 8
    out = ctx @ Wo.T + bo + x

Sharding: data-parallel over batch B=8 — one batch element per NeuronCore,
no collectives. All matmul operands are bf16 (PSUM accumulation stays f32).

Single pass over the 32 token tiles (vs. the old two-pass + DRAM spill):
  - x^T comes straight from DRAM through the DMA xbar transpose (one
    InstDmaTransposeAnt per tile) — no PE transposes, no PSUM copies.
  - LayerNorm is folded into the Q projection:
        q = rstd * (x @ wqt - mu x rw) + bq_eff,   rw_j = sum_i wqt[i,j]
    mu enters as a rank-1 matmul (mu row x -rw row); rstd enters as the
    per-partition *scale* operand of the ELU's two Relu activations.
    The mu row itself is a ones-column matmul against x^T, borrowed into
    the q0 PSUM bank before the q accumulation starts.
  - elu(x)+1 == relu(x) + exp(min(x,0)) exactly, so the ELU is
    relu(-x) -> exp(-.) on ACT, relu(x) on ACT, one add on Pool; the
    "-1" folds into Square's bias (sum-of-squares) and a tensor_scalar.
  - all rsqrt via exp(-0.5*ln(.)) on ACT: ln and exp live in the same
    activation table set (natural_log_exp_and_others) so there are no
    1.28us table swaps, and no DVE Newton chains.
  - the k-side l2 norm is applied to V instead of K (kv = sum ek x v is
    bilinear), saving a broadcast multiply from PSUM.
  - per-head kv state accumulates into a single PSUM bank: even heads at
    partitions 0-63, odd heads at 64-127 via tile_position=(0,64).
  - eq is written bf16 and xbar-transposed SBUF->SBUF into a persistent
    [128, 8, 4096] eqT buffer.
  - the per-head state is accumulated TRANSPOSED (vk_h = sum_s v x ek)
    so that pass 2 can fold the whole attention tail into one GEMM:
        out = eq @ M + x,   M = blockdiag(kv_h) @ wot
    M is built with 32 small matmuls once per iteration; this removes
    the per-chunk ctx matmuls and all 256 ctx^T PSUM->SBUF copies.
  - vk accumulation matmuls for tile t issue at the start of tile t+1
    so the PE never waits on the k-norm chain.

Biases are all zero for this problem's inputs; a with_bias variant adds
rank-1 (ones x bias) matmuls into each projection's PSUM group.
"""

import contextlib

import numpy as np

import concourse.bass as bass
import concourse.mybir as mybir
import concourse.tile as tile
from concourse import bacc
from concourse.masks import make_identity

B, S, HID = 8, 4096, 1024
NH, HD = 16, 64
P = 128
NT = S // P            # 32 token tiles
NC = HID // P          # 8 feature chunks
CHUNK = 4              # token tiles per ctx chunk (512 tokens)
NCHUNKS = NT // CHUNK
LN_EPS = 1e-12

F32 = mybir.dt.float32
BF16 = mybir.dt.bfloat16
AF = mybir.ActivationFunctionType
OP = mybir.AluOpType
AX = mybir.AxisListType


def build_nc(with_bias=False, loop_n=1):
    nc = bacc.Bacc("TRN2", target_bir_lowering=False, enable_partition_id=False)

    x_d = nc.dram_tensor("x", [S, HID], BF16, kind="ExternalInput")
    wqt_d = nc.dram_tensor("wqt", [HID, HID], BF16, kind="ExternalInput")
    wkt_d = nc.dram_tensor("wkt", [HID, HID], BF16, kind="ExternalInput")
    wvt_d = nc.dram_tensor("wvt", [HID, HID], BF16, kind="ExternalInput")
    wot_d = nc.dram_tensor("wot", [HID, HID], BF16, kind="ExternalInput")
    nrw_d = nc.dram_tensor("nrw", [1, HID], BF16, kind="ExternalInput")
    bias_d = {}
    if with_bias:
        for nm in ("bq", "bk", "bv", "bo"):
            bias_d[nm] = nc.dram_tensor(nm, [1, HID], BF16, kind="ExternalInput")
    out_d = nc.dram_tensor("out", [S, HID], F32, kind="ExternalOutput")

    with tile.TileContext(nc) as tc, contextlib.ExitStack() as ctx:
        persist = ctx.enter_context(tc.tile_pool(name="persist", bufs=1))
        wpool = ctx.enter_context(tc.tile_pool(name="weights", bufs=1))

        ident = persist.tile([P, P], F32)
        make_identity(nc, ident)
        eps_c = persist.tile([P, 1], F32)
        nc.gpsimd.memset(eps_c, LN_EPS)
        negone_c = persist.tile([P, 1], F32)
        nc.gpsimd.memset(negone_c, -1.0)
        kv_sb = persist.tile([P, NC, P], BF16)        # packed vk state
        eqT = persist.tile([P, NC, S], BF16)          # transposed eq, full S
        nrw_sb = persist.tile([1, HID], BF16)
        nc.sync.dma_start(nrw_sb, nrw_d.ap())
        brow = {}
        if with_bias:
            ones_row = persist.tile([1, P], BF16)
            nc.gpsimd.memset(ones_row, 1.0)
            for nm, d in bias_d.items():
                t_ = persist.tile([1, HID], BF16, name=f"{nm}_row")
                nc.sync.dma_start(t_, d.ap())
                brow[nm] = t_

        # loop-invariant weight loads (once per execution, not per iteration)
        wk_sb = wpool.tile([P, NC, HID], BF16, tag="wA", name="wk_sb")
        nc.gpsimd.dma_start(wk_sb, wkt_d.ap().rearrange("(c p) j -> p c j", p=P))
        wv_sb = wpool.tile([P, NC, HID], BF16, tag="wB", name="wv_sb")
        nc.gpsimd.dma_start(wv_sb, wvt_d.ap().rearrange("(c p) j -> p c j", p=P))
        wq_sb = wpool.tile([P, NC, HID], BF16, tag="wC", name="wq_sb")
        nc.gpsimd.dma_start(wq_sb, wqt_d.ap().rearrange("(c p) j -> p c j", p=P))
        wo_sb = wpool.tile([P, NC, HID], BF16, tag="wD", name="wo_sb")
        nc.gpsimd.dma_start(wo_sb, wot_d.ap().rearrange("(c p) j -> p c j", p=P))

        _loop = tc.For_i(0, loop_n, 1) if loop_n > 1 else contextlib.nullcontext(0)
        with _loop:
            # ------------- pass 1: k/v/q, kv state, eqT -------------

            with tc.tile_pool(name="sbufA", bufs=1) as sa, \
                 tc.tile_pool(name="psumA", bufs=1, space="PSUM") as pa:
                # vk state per head pair a: kv_ps[:, a, :] = [vs_2a|vs_2a+1]^T
                # @ [ek_2a|ek_2a+1]; diagonal 64x64 blocks are vk_2a / vk_2a+1,
                # off-diagonal cross-head blocks are never read. Explicitly
                # zeroed (start=True pend-zero covers only the issuing
                # matmul's col-strip, and PSUM persists across executions).
                kv_ps = pa.tile([P, NC, P], F32, tag="kv", name="kv_ps")
                nc.vector.memset(kv_ps, 0.0)
                pend_kv = []           # deferred kv matmuls (ek, vs) per tile

                def flush_kv(last):
                    if not pend_kv:
                        return
                    first, ek, vs = pend_kv.pop()
                    ekf = ek[:].rearrange("p h d -> p (h d)")
                    vsf = vs[:].rearrange("p h d -> p (h d)")
                    for a in range(NC):
                        nc.tensor.matmul(
                            kv_ps[:, a, :],
                            vsf[:, a * P:(a + 1) * P],
                            ekf[:, a * P:(a + 1) * P],
                            start=False, stop=last, skip_group_check=True)

                for t in range(NT):
                    tok = slice(t * P, (t + 1) * P)
                    x_t = sa.tile([P, HID], BF16, tag="x", bufs=3, name=f"x_{t}")
                    nc.scalar.dma_start(x_t, x_d.ap()[tok, :])
                    xT = sa.tile([P, NC, P], BF16, tag="xT", bufs=3,
                                 name=f"xT_{t}")
                    nc.sync.dma_start_transpose(xT, x_d.ap()[tok, :])

                    # LayerNorm stats; rstd = exp(-0.5*ln(var+eps))
                    stats = sa.tile([P, 2, 6], F32, tag="st", bufs=3,
                                    name=f"st_{t}")
                    xg = x_t[:].rearrange("p (g d) -> p g d", g=2)
                    for g in range(2):
                        nc.vector.bn_stats(stats[:, g, :], xg[:, g, :])
                    mv = sa.tile([P, 2], F32, tag="mv", bufs=3, name=f"mv_{t}")
                    nc.vector.bn_aggr(mv, stats)
                    lnv = sa.tile([P, 1], F32, tag="lnv", bufs=3, name=f"lnv_{t}")
                    nc.scalar.activation(lnv, mv[:, 1:2], AF.Ln, bias=eps_c[:, 0:1])
                    rstd = sa.tile([P, 1], F32, tag="rsd", bufs=3, name=f"rsd_{t}")
                    nc.scalar.activation(rstd, lnv, AF.Exp, scale=-0.5)
                    nrstd = sa.tile([P, 1], F32, tag="nrs", bufs=3, name=f"nrs_{t}")
                    nc.vector.tensor_scalar(nrstd, rstd, -1.0, None, OP.mult)

                    def proj(ps, w_sb, half, extras=()):
                        sl = slice(half * 512, (half + 1) * 512)
                        for c in range(NC):
                            nc.tensor.matmul(
                                ps, xT[:, c, :], w_sb[:, c, sl],
                                start=(c == 0),
                                stop=(c == NC - 1 and not extras),
                                skip_group_check=True)
                        for i, (lhs, rhs_row) in enumerate(extras):
                            nc.tensor.matmul(
                                ps, lhs, rhs_row[0:1, sl],
                                start=False, stop=(i == len(extras) - 1),
                                skip_group_check=True)

                    # kv accumulation for the previous tile (PE never waits)
                    flush_kv(last=False)

                    # ---- K ----
                    k_ps = [pa.tile([P, 512], F32, tag="kh", bufs=2,
                                    name=f"k_ps{t}_{h}") for h in range(2)]
                    kex = [(ones_row, brow["bk"])] if with_bias else []
                    w1k = sa.tile([P, HID], BF16, tag="w1k", bufs=3,
                                  name=f"w1k_{t}")
                    for half in range(2):
                        proj(k_ps[half], wk_sb, half, kex)
                        hs = slice(half * 512, (half + 1) * 512)
                        r = sa.tile([P, 512], BF16, tag="kr", bufs=3,
                                    name=f"kr_{t}_{half}")
                        nc.scalar.activation(r, k_ps[half], AF.Relu, scale=-1.0)
                        e = sa.tile([P, 512], BF16, tag="ke", bufs=3,
                                    name=f"ke_{t}_{half}")
                        nc.scalar.activation(e, r, AF.Exp, scale=-1.0)
                        m = sa.tile([P, 512], BF16, tag="km", bufs=3,
                                    name=f"km_{t}_{half}")
                        nc.scalar.activation(m, k_ps[half], AF.Relu)
                        nc.gpsimd.tensor_tensor(w1k[:, hs], m, e, OP.add)
                    sqk = sa.tile([P, NH, HD], BF16, tag="sqk", bufs=2,
                                  name=f"sqk_{t}")
                    nc.scalar.activation(
                        sqk[:].rearrange("p h d -> p (h d)"), w1k, AF.Square,
                        bias=negone_c[:, 0:1])
                    ssk = sa.tile([P, NH], F32, tag="ssk", bufs=3,
                                  name=f"ssk_{t}")
                    nc.vector.tensor_reduce(ssk, sqk, AX.X, OP.add)
                    lnk = sa.tile([P, NH], F32, tag="lnk", bufs=3,
                                  name=f"lnk_{t}")
                    nc.scalar.activation(lnk, ssk, AF.Ln)
                    rnk = sa.tile([P, NH], F32, tag="rnk", bufs=3,
                                  name=f"rnk_{t}")
                    nc.scalar.activation(rnk, lnk, AF.Exp, scale=-0.5)
                    ek = sa.tile([P, NH, HD], BF16, tag="ek", bufs=2,
                                 name=f"ek_{t}")
                    nc.vector.tensor_scalar(
                        ek[:].rearrange("p h d -> p (h d)"), w1k, 1.0, None,
                        OP.subtract)

                    # ---- V (k-norm folded in) ----
                    v_ps = [pa.tile([P, 512], F32, tag="vh", bufs=2,
                                    name=f"v_ps{t}_{h}") for h in range(2)]
                    vex = [(ones_row, brow["bv"])] if with_bias else []
                    vs = sa.tile([P, NH, HD], BF16, tag="vs", bufs=2,
                                 name=f"vs_{t}")
                    for half in range(2):
                        proj(v_ps[half], wv_sb, half, vex)
                        hh = slice(half * 8, (half + 1) * 8)
                        nc.vector.tensor_tensor(
                            vs[:, hh, :],
                            v_ps[half][:].rearrange("p (h d) -> p h d", d=HD),
                            rnk[:, hh, None].to_broadcast((P, 8, HD)),
                            OP.mult)

                    # mu row via one PE transpose of [mu, var], borrowed
                    # into the q0 psum bank before the q accumulation
                    q_ps = [pa.tile([P, 512], F32, tag="qh", bufs=2,
                                    name=f"q_ps{t}_{h}") for h in range(2)]
                    nc.tensor.transpose(q_ps[0][0:2, 0:P], mv, ident)
                    mu_row = sa.tile([1, P], BF16, tag="mu", bufs=3,
                                     name=f"mu_{t}")
                    nc.vector.tensor_scalar(mu_row, q_ps[0][0:1, 0:P],
                                            1.0, None, OP.mult)

                    # ---- Q (LayerNorm folded in) ----
                    qex = [(mu_row, nrw_sb)]
                    if with_bias:
                        qex.append((ones_row, brow["bq"]))
                    w1q = sa.tile([P, HID], BF16, tag="w1q", bufs=3,
                                  name=f"w1q_{t}")
                    for half in range(2):
                        proj(q_ps[half], wq_sb, half, qex)
                        hs = slice(half * 512, (half + 1) * 512)
                        r = sa.tile([P, 512], BF16, tag="qr", bufs=3,
                                    name=f"qr_{t}_{half}")
                        nc.scalar.activation(r, q_ps[half], AF.Relu,
                                             scale=nrstd[:, 0:1])
                        e = sa.tile([P, 512], BF16, tag="qe", bufs=3,
                                    name=f"qe_{t}_{half}")
                        nc.scalar.activation(e, r, AF.Exp, scale=-1.0)
                        m = sa.tile([P, 512], BF16, tag="qm", bufs=3,
                                    name=f"qm_{t}_{half}")
                        nc.scalar.activation(m, q_ps[half], AF.Relu,
                                             scale=rstd[:, 0:1])
                        nc.gpsimd.tensor_tensor(w1q[:, hs], m, e, OP.add)
                    sqq = sa.tile([P, NH, HD], BF16, tag="sqq", bufs=2,
                                  name=f"sqq_{t}")
                    nc.scalar.activation(
                        sqq[:].rearrange("p h d -> p (h d)"), w1q, AF.Square,
                        bias=negone_c[:, 0:1])
                    ssq = sa.tile([P, NH], F32, tag="ssq", bufs=3,
                                  name=f"ssq_{t}")
                    nc.vector.tensor_reduce(ssq, sqq, AX.X, OP.add)
                    lnq = sa.tile([P, NH], F32, tag="lnq", bufs=3,
                                  name=f"lnq_{t}")
                    nc.scalar.activation(lnq, ssq, AF.Ln)
                    rnq = sa.tile([P, NH], BF16, tag="rnq", bufs=3,
                                  name=f"rnq_{t}")
                    nc.scalar.activation(rnq, lnq, AF.Exp, scale=-0.5)
                    eqp = sa.tile([P, NH, HD], BF16, tag="eqp", bufs=2,
                                  name=f"eqp_{t}")
                    nc.vector.tensor_scalar(
                        eqp[:].rearrange("p h d -> p (h d)"), w1q, 1.0, None,
                        OP.subtract)
                    eq = sa.tile([P, NH, HD], BF16, tag="eq", bufs=2,
                                 name=f"eq_{t}")
                    nc.vector.tensor_tensor(
                        eq, eqp, rnq[:, :, None].to_broadcast((P, NH, HD)),
                        OP.mult)
                    nc.sync.dma_start_transpose(
                        eqT[:, :, tok], eq[:].rearrange("p h d -> p (h d)"))

                    pend_kv.append((t == 0, ek, vs))

                flush_kv(last=True)
                for a in range(NC):
                    nc.vector.tensor_copy(kv_sb[:, a, :], kv_ps[:, a, :])

            # ------------- pass 2: out = eq @ (blockdiag(kv) @ wot) + x --
            with tc.tile_pool(name="sbufB", bufs=1) as sb, \
                 tc.tile_pool(name="psumB", bufs=1, space="PSUM") as pb:
                # M[c-chunk][p, j]: rows = (head, d) feature index c*128+p.
                # vk_sb holds per-head vk[e, d]: head 2a at partitions 0-63,
                # col block a; head 2a+1 at partitions 64-127, col block a.
                m_sb = sb.tile([P, NC, HID], BF16, tag="m", bufs=1, name="m_sb")
                for c in range(NC):
                    for half in range(2):
                        hs = slice(half * 512, (half + 1) * 512)
                        m_pse = pb.tile([HD, 512], F32, tag="mpse", bufs=2,
                                        name=f"m_pse{c}_{half}")
                        m_pso = pb.tile([HD, 512], F32, tag="mpso", bufs=2,
                                        name=f"m_pso{c}_{half}")
                        nc.tensor.matmul(
                            m_pse, kv_sb[0:HD, c, 0:HD],
                            wo_sb[0:HD, c, hs], start=True, stop=True)
                        nc.tensor.matmul(
                            m_pso, kv_sb[HD:P, c, HD:P],
                            wo_sb[HD:P, c, hs], start=True, stop=True)
                        nc.scalar.copy(m_sb[0:HD, c, hs], m_pse)
                        nc.scalar.copy(m_sb[HD:P, c, hs], m_pso)

                for t in range(NT):
                    tok = slice(t * P, (t + 1) * P)
                    x2 = sb.tile([P, HID], BF16, tag="x2", bufs=2,
                                 name=f"x2_{t}")
                    nc.scalar.dma_start(x2, x_d.ap()[tok, :])
                    out_sb = sb.tile([P, HID], F32, tag="osb", bufs=2,
                                     name=f"out_{t}")
                    for half in range(2):
                        hs = slice(half * 512, (half + 1) * 512)
                        o_ps = pb.tile([P, 512], F32, tag="oh", bufs=3,
                                       name=f"o_ps{t}_{half}")
                        for c in range(NC):
                            nc.tensor.matmul(
                                o_ps, eqT[:, c, tok], m_sb[:, c, hs],
                                start=(c == 0),
                                stop=(c == NC - 1 and not with_bias),
                                skip_group_check=True)
                        if with_bias:
                            nc.tensor.matmul(
                                o_ps, ones_row, brow["bo"][0:1, hs],
                                start=False, stop=True,
                                skip_group_check=True)
                        nc.vector.tensor_tensor(
                            out_sb[:, hs], o_ps, x2[:, hs], OP.add)
                    nc.gpsimd.dma_start(out_d.ap()[tok, :], out_sb)

    nc.compile()
    _force_single_act_table(nc)
    return nc


def _force_single_act_table(nc):
    """Retarget all activation-table loads to natural_log_exp_and_others
    (which contains every function this kernel uses: ln, exp, relu, square,
    copy) and drop the now-redundant reloads. The insertion pass picks the
    first table containing each function, which thrashes exp<->ln at 1.28us
    per reload, ~6x per token tile."""
    from concourse.hw_specs import get_activation_tables

    names = list(get_activation_tables(nc.m.arch).keys())
    target = names.index("natural_log_exp_and_others")
    kept_one = False
    for b in nc.main_func.blocks:
        keep = []
        for i in b.instructions:
            if isinstance(i, mybir.InstLoadActFuncSet):
                si = getattr(i, "sync_info", None)
                has_sync = si is not None and (
                    len(si.on_wait) > 0 or len(si.on_update) > 0)
                i.act_func_set_id = target
                if not kept_one or has_sync:
                    kept_one = True
                    keep.append(i)
            else:
                keep.append(i)
        b.instructions[:] = keep


_RUNNER = {}


def _get_runner(loop_n=1, with_bias=False):
    key = (loop_n, with_bias)
    if key in _RUNNER:
        return _RUNNER[key]

    import jax
    from jax.sharding import Mesh, PartitionSpec
    from jax.experimental.shard_map import shard_map
    from concourse.bass2jax import _bass_exec_p, install_neuronx_cc_hook

    install_neuronx_cc_hook()
    nc = build_nc(with_bias=with_bias, loop_n=loop_n)

    in_names = []
    out_names = []
    out_avals = []
    for alloc in nc.m.functions[0].allocations:
        if not isinstance(alloc, mybir.MemoryLocationSet):
            continue
        name = alloc.memorylocations[0].name
        if alloc.kind == "ExternalInput":
            in_names.append(name)
        elif alloc.kind == "ExternalOutput":
            out_names.append(name)
            out_avals.append(
                jax.core.ShapedArray(tuple(alloc.tensor_shape),
                                     mybir.dt.np(alloc.dtype)))
    n_params = len(in_names)
    all_in_names = in_names + out_names

    def _body(*args):
        outs = _bass_exec_p.bind(
            *args,
            out_avals=tuple(out_avals),
            in_names=tuple(all_in_names),
            out_names=tuple(out_names),
            lowering_input_output_aliases=(),
            sim_require_finite=True,
            sim_require_nnan=True,
            nc=nc,
        )
        return tuple(outs)

    devices = jax.devices()[:B]
    mesh = Mesh(np.asarray(devices), ("core",))
    n_outs = len(out_names)
    fn = jax.jit(
        shard_map(
            _body, mesh=mesh,
            in_specs=(PartitionSpec("core"),) * (n_params + n_outs),
            out_specs=(PartitionSpec("core"),) * n_outs,
            check_rep=False,
        ),
        keep_unused=True,
    )
    _RUNNER[key] = (fn, in_names, out_names, out_avals)
    return _RUNNER[key]


def prep_inputs(input_tensor, attention_mask, ln_gamma, ln_beta,
                Wq, bq, Wk, bk, Wv, bv, Wo, bo):
    """Host-side static prep: transpose weights, fold gamma/beta/scale."""
    import ml_dtypes
    bf = ml_dtypes.bfloat16
    f = np.float32
    x = np.asarray(input_tensor, f)
    g = np.asarray(ln_gamma, f)
    be = np.asarray(ln_beta, f)
    Wq = np.asarray(Wq, f); Wk = np.asarray(Wk, f)
    Wv = np.asarray(Wv, f); Wo = np.asarray(Wo, f)
    wqt = np.ascontiguousarray((Wq * g[None, :]).T)        # [i, j]
    nrw = -wqt.sum(axis=0, keepdims=True)                  # [1, j]
    bq_eff = (np.asarray(bq, f) + be @ Wq.T).astype(f)
    per_core = {
        "wqt": wqt.astype(bf),
        "wkt": np.ascontiguousarray(Wk.T).astype(bf),
        "wvt": np.ascontiguousarray(Wv.T).astype(bf),
        "wot": np.ascontiguousarray(Wo.T * np.float32(1.0 / np.sqrt(HD))).astype(bf),
        "nrw": nrw.astype(bf),
    }
    biases = {"bq": bq_eff, "bk": np.asarray(bk, f),
              "bv": np.asarray(bv, f), "bo": np.asarray(bo, f)}
    has_bias = any(np.any(v) for v in biases.values())
    if has_bias:
        for nm, v in biases.items():
            per_core[nm] = v.reshape(1, HID).astype(bf)
    return np.ascontiguousarray(x.astype(bf)), per_core, has_bias


def kernel(**inputs) -> np.ndarray:
    x, per_core, has_bias = prep_inputs(**inputs)
    fn, in_names, out_names, out_avals = _get_runner(1, has_bias)

    concat_in = []
    for name in in_names:
        if name == "x":
            concat_in.append(x.reshape(B * S, HID))
        else:
            concat_in.append(np.concatenate([per_core[name]] * B, axis=0))
    concat_zeros = [
        np.zeros((B * av.shape[0], *av.shape[1:]), av.dtype) for av in out_avals
    ]
    out_arrs = fn(*concat_in, *concat_zeros)
    out = np.asarray(out_arrs[out_names.index("out")])
    return out.reshape(B, S, HID)
